# revision 30
# baseline (speedup 1.0000x reference)
"""AttCML distributed Bass kernel for 8 TRN2 NeuronCores.

Sharding: data-parallel over the batch dim (16384 / 8 = 2048 per core).

The on-device toolchain here has no usable wide-index row gather
(indirect DMA is not lowered by this walrus pipeline; the Q7 dma_gather
ucode is int16-indexed), so kernel() performs the embedding-row lookup
host-side and ships packed per-core tensors; all attention compute
(scores, exp/normalize, weighted pooling, distances) runs on device.

Device-side structure per 128-batch tile (batch on partitions):
  - both targets (pos/neg) processed in single wide ops over a
    [128, 2, C, D] layout to halve instruction count
  - dot-product and pooling reductions are done as TT-add trees, which
    run at the DVE bf16 2x packed rate (native tensor_reduce runs 1x)
  - masked-out pref slots (position >= n+1) are never shipped: the host
    packs only cap[t] slots per tile and pads with the zero row; the
    kernel subtracts the pad count from the softmax denominator, and
    zero rows contribute nothing to the weighted sum — exact semantics.

Per-core batch layout: batch element order[p * NT + t] is at partition p,
tile t. The host unscrambles at the end.
"""

import numpy as np
from contextlib import ExitStack

try:
    import concourse  # noqa: F401
except ImportError:  # pragma: no cover
    import sys

    for _p in ("/opt/trn_rl_repo", "/root/.axon_site/_ro/trn_rl_repo"):
        if _p not in sys.path:
            sys.path.insert(0, _p)

import concourse.bacc as bacc
import concourse.tile as tile
from concourse import mybir
from concourse.bass_utils import run_bass_kernel_spmd

F32 = mybir.dt.float32
BF16 = mybir.dt.bfloat16
ALU = mybir.AluOpType
AXIS = mybir.AxisListType
ACTF = mybir.ActivationFunctionType

D = 128          # embedding dim
P = 50           # prefs per batch element
N_CORES = 8
B = 16384
BC = B // N_CORES  # 2048 batch per core
PB = 128           # batch tile = one SBUF partition set


def build_bass(bc: int = BC, cap=None):
    nt = bc // PB
    if cap is None:
        cap = (P,) * nt
    assert len(cap) == nt
    offs = [0]
    for c in cap:
        offs.append(offs[-1] + c)
    ctot = offs[-1]

    nc = bacc.Bacc(
        "TRN2",
        target_bir_lowering=False,
        debug=False,
        enable_asserts=False,
        num_devices=N_CORES,
    )

    # pref rows, bf16, host-packed: [PB, sum(cap) * D]
    pref_in = nc.declare_dram_parameter("pref", [PB, ctot * D], BF16, isOutput=False)
    # u vectors f32 [PB, nt*D]; pos/neg packed together [PB, nt*2*D]
    u_in = nc.declare_dram_parameter("uvec", [PB, nt * D], F32, isOutput=False)
    pn_in = nc.declare_dram_parameter("pnvec", [PB, nt * 2 * D], F32, isOutput=False)
    padc_in = nc.declare_dram_parameter("padc", [PB, nt], F32, isOutput=False)
    out = nc.declare_dram_parameter("out", [PB, 2 * nt], F32, isOutput=True)

    with tile.TileContext(nc) as tc, ExitStack() as ctx:
        consts = ctx.enter_context(tc.tile_pool(name="consts", bufs=1))
        pref_pool = ctx.enter_context(tc.tile_pool(name="pref", bufs=2))
        tmp_pool = ctx.enter_context(tc.tile_pool(name="tmpp", bufs=2))
        big_pool = ctx.enter_context(tc.tile_pool(name="big", bufs=1))
        vec_pool = ctx.enter_context(tc.tile_pool(name="vec", bufs=2))
        small_pool = ctx.enter_context(tc.tile_pool(name="small", bufs=3))

        padc = consts.tile([PB, nt], F32)
        nc.sync.dma_start(padc[:], padc_in[:])
        res = consts.tile([PB, 2 * nt], F32)

        for t in range(nt):
            C = cap[t]
            L = C * D
            pref = pref_pool.tile([PB, L], BF16, tag="pref")
            nc.sync.dma_start(pref[:], pref_in[:, offs[t] * D : offs[t + 1] * D])

            u_t = vec_pool.tile([PB, D], F32, tag="u")
            nc.sync.dma_start(u_t[:], u_in[:, t * D : (t + 1) * D])
            pn_t = vec_pool.tile([PB, 2 * D], F32, tag="pn")
            nc.sync.dma_start(pn_t[:], pn_in[:, t * 2 * D : (t + 1) * 2 * D])

            # bf16 copy of both targets (ACT)
            pn_b = vec_pool.tile([PB, 2 * D], BF16, tag="pnb")
            nc.scalar.copy(pn_b[:], pn_t[:])

            # ---- stage A: w[b, s, j] = pref[b, j, :] . tgt[b, s, :] ----
            # per-target ops keep the plain 3D broadcast form that measures
            # at the full 2x packed rate
            tmp = tmp_pool.tile([PB, 2 * L], BF16, tag="tmp")
            prefA = pref[:].rearrange("p (j d) -> p j d", d=D)
            for s in range(2):
                nc.vector.tensor_tensor(
                    out=tmp[:, s * L : (s + 1) * L].rearrange(
                        "p (j d) -> p j d", d=D
                    ),
                    in0=prefA,
                    in1=pn_b[:, s * D : (s + 1) * D]
                    .rearrange("p (o d) -> p o d", o=1)
                    .to_broadcast([PB, C, D]),
                    op=ALU.mult,
                )

            # tree-reduce over d at the TT 2x bf16 rate; short 1x tail
            cur, dd, lvl = tmp, D, 0
            while dd > 4:
                h = dd // 2
                nxt = big_pool.tile([PB, 2 * C * h], BF16, tag=f"ar{lvl}")
                lvl += 1
                nc.vector.tensor_tensor(
                    out=nxt[:].rearrange("p (k d) -> p k d", d=h),
                    in0=cur[:].rearrange("p (k d) -> p k d", d=dd)[:, :, :h],
                    in1=cur[:].rearrange("p (k d) -> p k d", d=dd)[:, :, h:],
                    op=ALU.add,
                )
                cur, dd = nxt, h
            w2 = small_pool.tile([PB, 2 * P], F32, tag="w2")
            nc.vector.tensor_reduce(
                out=w2[:, : 2 * C],
                in_=cur[:].rearrange("p (k d) -> p k d", d=dd),
                axis=AXIS.X,
                op=ALU.add,
            )

            # ---- stage B: att = exp(w) / (sum - padcount) ----
            e2 = small_pool.tile([PB, 2 * P], F32, tag="e2")
            nc.scalar.activation(e2[:, : 2 * C], w2[:, : 2 * C], ACTF.Exp)
            ssum = small_pool.tile([PB, 2], F32, tag="ssum")
            nc.vector.tensor_reduce(
                ssum[:],
                e2[:, : 2 * C].rearrange("p (s j) -> p s j", s=2),
                axis=AXIS.X,
                op=ALU.add,
            )
            scor = small_pool.tile([PB, 2], F32, tag="scor")
            nc.vector.tensor_tensor(
                scor[:],
                ssum[:],
                padc[:, t : t + 1].to_broadcast([PB, 2]),
                op=ALU.subtract,
            )
            rs = small_pool.tile([PB, 2], F32, tag="rs")
            nc.vector.reciprocal(rs[:], scor[:])
            att2 = small_pool.tile([PB, 2 * P], BF16, tag="att2")
            nc.vector.tensor_tensor(
                out=att2[:, : 2 * C].rearrange("p (s j) -> p s j", s=2),
                in0=e2[:, : 2 * C].rearrange("p (s j) -> p s j", s=2),
                in1=rs[:].rearrange("p (s o) -> p s o", o=1).to_broadcast(
                    [PB, 2, C]
                ),
                op=ALU.mult,
            )

            # ---- stage C: r[b, s, :] = sum_j att[b, s, j] * pref[b, j, :] ----
            # chunk expansion (ACT) + multiply (DVE) by target so the two
            # engines pipeline instead of DVE stalling on the full expansion
            attx = big_pool.tile([PB, 2 * L], BF16, tag="attx")
            tmp2 = big_pool.tile([PB, 2 * L], BF16, tag="tmp2")
            halves = [(0, C // 2), (C // 2, C)] if C >= 24 else [(0, C)]
            for s in range(2):
                for j0, j1 in halves:
                    cw = j1 - j0
                    lo = s * L + j0 * D
                    hi = s * L + j1 * D
                    attx_s = attx[:, lo:hi].rearrange("p (j d) -> p j d", d=D)
                    nc.scalar.copy(
                        attx_s,
                        att2[:, s * C + j0 : s * C + j1]
                        .rearrange("p (j o) -> p j o", o=1)
                        .to_broadcast([PB, cw, D]),
                    )
                    nc.vector.tensor_tensor(
                        out=tmp2[:, lo:hi].rearrange("p (j d) -> p j d", d=D),
                        in0=pref[:, j0 * D : j1 * D].rearrange(
                            "p (j d) -> p j d", d=D
                        ),
                        in1=attx_s,
                        op=ALU.mult,
                    )
            # tree-reduce over j (both targets at once)
            cur, cj, lvl = tmp2, C, 0
            while cj > 4:
                h = cj // 2
                odd = cj - 2 * h
                nxt = big_pool.tile([PB, 2 * (h + odd) * D], BF16, tag=f"cr{lvl}")
                lvl += 1
                nxt4 = nxt[:].rearrange("p (s j d) -> p s j d", s=2, d=D)
                cur4 = cur[:].rearrange("p (s j d) -> p s j d", s=2, d=D)
                nc.vector.tensor_tensor(
                    out=nxt4[:, :, :h, :],
                    in0=cur4[:, :, :h, :],
                    in1=cur4[:, :, h : 2 * h, :],
                    op=ALU.add,
                )
                if odd:
                    nc.vector.tensor_copy(
                        nxt4[:, :, h:, :], cur4[:, :, 2 * h : cj, :]
                    )
                cur, cj = nxt, h + odd
            r2 = vec_pool.tile([PB, 2 * D], F32, tag="r2")
            nc.vector.tensor_reduce(
                r2[:].rearrange("p (s d) -> p s d", s=2),
                cur[:, : 2 * cj * D].rearrange("p (s j d) -> p s d j", s=2, d=D),
                axis=AXIS.X,
                op=ALU.add,
            )

            # ---- distances: ||u + r - tgt||^2 ----
            du2 = vec_pool.tile([PB, 2 * D], F32, tag="du2")
            nc.vector.tensor_tensor(
                du2[:].rearrange("p (s d) -> p s d", s=2),
                u_t[:].rearrange("p (o d) -> p o d", o=1).to_broadcast(
                    [PB, 2, D]
                ),
                pn_t[:].rearrange("p (s d) -> p s d", s=2),
                op=ALU.subtract,
            )
            diff2 = vec_pool.tile([PB, 2 * D], F32, tag="diff2")
            nc.vector.tensor_add(diff2[:], r2[:], du2[:])
            sq2 = vec_pool.tile([PB, 2 * D], F32, tag="sq2")
            nc.scalar.square(sq2[:], diff2[:])
            nc.vector.tensor_reduce(
                out=res[:].rearrange("p (s q) -> p s q", q=nt)[:, :, t : t + 1],
                in_=sq2[:].rearrange("p (s d) -> p s d", s=2),
                axis=AXIS.X,
                op=ALU.add,
            )

        nc.sync.dma_start(out[:], res[:])

    nc.compile()
    return nc


_CACHE: dict = {}


def _get_bass(bc: int, cap: tuple):
    key = (bc, cap)
    if key not in _CACHE:
        _CACHE[key] = build_bass(bc, cap)
    return _CACHE[key]


def prep_core(user_emb, ctx_item_bf16, ctx_item, user_ids, pos_ids, neg_ids,
              pref_ids, n_prefs, cap, order):
    """Build one core's input map.

    order: [bc] permutation; element order[p * nt + t] is placed at
    partition p, tile t.  Host guarantees n_prefs[order[p*nt+t]] + 1 <= cap[t].
    """
    bc = order.shape[0]
    nt = bc // PB
    offs = np.concatenate([[0], np.cumsum(cap)]).astype(np.int64)
    ctot = int(offs[-1])

    ob = order.reshape(PB, nt)
    n1 = (n_prefs[ob] + 1.0).astype(np.float32)  # [PB, nt] valid counts

    pref = np.zeros((PB, ctot, D), dtype=ctx_item_bf16.dtype)
    for t in range(nt):
        C = int(cap[t])
        ids_t = pref_ids[ob[:, t], :C].copy()  # [PB, C]
        slot = np.arange(C)[None, :]
        ids_t[slot >= n1[:, t : t + 1]] = ctx_item_bf16.shape[0] - 1
        pref[:, offs[t] : offs[t + 1], :] = ctx_item_bf16[ids_t]

    uvec = user_emb[user_ids[ob].reshape(-1)].reshape(PB, nt * D)
    pn = np.empty((PB, nt, 2, D), np.float32)
    pn[:, :, 0, :] = ctx_item[pos_ids[ob]]
    pn[:, :, 1, :] = ctx_item[neg_ids[ob]]
    padc = (np.asarray(cap, np.float32)[None, :] - n1).astype(np.float32)

    return {
        "pref": np.ascontiguousarray(pref.reshape(PB, ctot * D)),
        "uvec": np.ascontiguousarray(uvec.astype(np.float32)),
        "pnvec": np.ascontiguousarray(pn.reshape(PB, nt * 2 * D)),
        "padc": padc,
    }


def plan_order(n_prefs_core, cap):
    """Assign the core's bc elements to (partition, tile) slots so each
    element lands in a tile with cap >= n+1. Returns order [bc] or None."""
    bc = n_prefs_core.shape[0]
    nt = bc // PB
    idx = np.argsort(n_prefs_core, kind="stable")  # ascending n
    order = np.empty(bc, dtype=np.int64)
    tile_order = np.argsort(np.asarray(cap), kind="stable")
    ok = True
    pos = 0
    for t in tile_order:
        members = idx[pos : pos + PB]
        if (n_prefs_core[members] + 1 > cap[t]).any():
            ok = False
        order[t::nt] = members
        pos += PB
    if not ok:
        return None
    return order


def default_caps(nt):
    # quantiles of Uniform{1..49} n_prefs + slack, rounded up to x4 for
    # clean reduction trees; DESCENDING so the big tiles start first and
    # the small ones fill the pipeline tail
    qs = []
    for i in range(nt):
        c = int(np.ceil(2 + 48.0 * (i + 1) / nt)) + 2
        c = min(P, ((c + 3) // 4) * 4)
        qs.append(c)
    return tuple(qs)


def kernel(user_emb, item_emb, user_ids, pos_ids, neg_ids, pref_ids, n_prefs,
           _trace=False):
    user_emb = np.ascontiguousarray(np.asarray(user_emb, np.float32))
    item_emb = np.asarray(item_emb, np.float32)
    ctx_item = np.concatenate([item_emb, np.zeros((1, D), np.float32)], axis=0)
    import ml_dtypes

    ctx_item_bf16 = ctx_item.astype(ml_dtypes.bfloat16)

    user_ids = np.asarray(user_ids)
    pos_ids = np.asarray(pos_ids)
    neg_ids = np.asarray(neg_ids)
    pref_ids = np.asarray(pref_ids)
    n_prefs = np.asarray(n_prefs, np.float32)

    nt = BC // PB
    cap = default_caps(nt)

    orders = []
    feasible = True
    for c in range(N_CORES):
        sl = slice(c * BC, (c + 1) * BC)
        o = plan_order(n_prefs[sl], cap)
        if o is None:
            feasible = False
            break
        orders.append(o)
    if not feasible:
        cap = (P,) * nt
        orders = [plan_order(n_prefs[c * BC : (c + 1) * BC], cap) for c in range(N_CORES)]

    nc = _get_bass(BC, cap)

    in_maps = []
    for c in range(N_CORES):
        sl = slice(c * BC, (c + 1) * BC)
        in_maps.append(
            prep_core(
                user_emb,
                ctx_item_bf16,
                ctx_item,
                user_ids[sl],
                pos_ids[sl],
                neg_ids[sl],
                pref_ids[sl],
                n_prefs[sl],
                cap,
                orders[c],
            )
        )

    res = run_bass_kernel_spmd(
        nc, in_maps, core_ids=list(range(N_CORES)), trace=_trace
    )

    out = np.empty((2, B), dtype=np.float32)
    for c in range(N_CORES):
        r = np.asarray(res.results[c]["out"])  # [PB, 2*nt]
        r = r.reshape(PB, 2, nt)  # [p, s, t]
        flat = r.transpose(1, 0, 2).reshape(2, BC)  # [(s), p*nt+t]
        out[:, c * BC : (c + 1) * BC][:, orders[c]] = flat
    if _trace:
        return out, res
    return out
